# revision 15
# baseline (speedup 1.0000x reference)
"""MultiHeadGraphAttention Trainium2 kernel (pipelined v4).

Data-parallel over batch: core b computes batch element b (B=8, 8 cores).

Per-core math (one batch element, N=2048 nodes, U=256 units, H=8 heads, d=32):
  Q = x Wq, K = x Wk, V = x Wv
  sT[k,q]  = sum_d KT[d,k] QT[d,q]           (scores, transposed layout)
  e        = exp(sT/sqrt(d)) * adjT          (masked exp)
  ctxT[d,q] = sum_k V[k,d] e[k,q] ; Z[q] = sum_k e[k,q]
  out      = (ctxT/Z).T @ Wo + bo

Key structure:
  - (qc, kb) blocks of [128 keys x 512 queries]; per block 4 head-PAIR
    PSUM score tiles [128, 2x512] (2 banks, double-buffered) pipeline
    ACT exp / DVE mask / PE matmuls across pairs and iterations.
  - Pair (g, pi) covers heads 4g+pi and 4g+pi+2 so their fused PV+Z
    matmuls land at col positions 0 and 64 of one cps tile.
  - Z is FUSED into the PV matmul: stationary [ones | V_h | zeros31]
    (M=64) makes row 0 of each 64-row block the softmax denominator --
    no separate Z matmuls (eliminates 512 PE instructions).
  - Per-pair exp mode: 'A' = ACT exp + DVE mask multiply; 'S' = DVE-only
    Schraudolph (i16 = A*s + B from PSUM, bitcast to bf16 ~ exp) + mask.
    A rotating half-share of 'S' offloads ACT, the global bottleneck.
  - Host pre-transposes x/adj, converts everything to bf16, and permutes
    Wo into per-pair-tile chunks (wo4) matching the fused ctx layout.
"""

import sys

for p in ("/opt/trn_rl_repo",):
    if p not in sys.path:
        sys.path.insert(0, p)

from contextlib import ExitStack

import numpy as np
import ml_dtypes

import concourse.bass as bass
import concourse.mybir as mybir
import concourse.tile as tile
from concourse import bacc
from concourse.bass_utils import run_bass_kernel_spmd

B, N, U, H, D = 8, 2048, 256, 8, 32
NB = N // 128          # 16 key blocks of 128
QC = 4                 # q chunks
QW = N // QC           # 512 q per chunk
SCALE = 1.0 / np.sqrt(np.float32(D))
# Schraudolph bf16: bits(bf16(2^x)) ~= round(128*(x+127-0.04368))
SCH_A = 128.0 * float(SCALE) * 1.4426950408889634
SCH_B = (127.0 - 0.04368) * 128.0

f32 = mybir.dt.float32
bf16 = mybir.dt.bfloat16
i16 = mybir.dt.int16
EXP = mybir.ActivationFunctionType.Exp
MULT = mybir.AluOpType.mult
ADD = mybir.AluOpType.add


def pair_mode(qc, kb, p):
    """Engine path for masked-exp of head pair p of (qc,kb).

    Every second (qc,kb) sends one pair (rotating) to the DVE-Schraudolph
    path; the rest use ACT-exp + DVE-mask.
    """
    return "S" if (kb % 2 == 0 and p == (kb // 2) % 4) else "A"


def build_program():
    nc = bacc.Bacc("TRN2", target_bir_lowering=False, debug=False,
                   enable_asserts=False, num_devices=B)

    xT_d = nc.dram_tensor("xT", [U, N], bf16, kind="ExternalInput").ap()
    adjT_d = nc.dram_tensor("adjT", [N, N], bf16, kind="ExternalInput").ap()
    wq_d = nc.dram_tensor("Wq", [U, U], bf16, kind="ExternalInput").ap()
    wk_d = nc.dram_tensor("Wk", [U, U], bf16, kind="ExternalInput").ap()
    wv_d = nc.dram_tensor("Wv", [U, U], bf16, kind="ExternalInput").ap()
    wo4_d = nc.dram_tensor("wo4", [4 * 128, U], bf16, kind="ExternalInput").ap()
    bo_d = nc.dram_tensor("bo", [U], f32, kind="ExternalInput").ap()
    out_d = nc.dram_tensor("out", [N, U], f32, kind="ExternalOutput").ap()

    with tile.TileContext(nc) as tc:
        with ExitStack() as ctx:
            kernel_body(ctx, tc, xT_d, adjT_d, wq_d, wk_d, wv_d, wo4_d,
                        bo_d, out_d)
    nc.compile()
    return nc


def kernel_body(ctx, tc, xT_d, adjT_d, wq_d, wk_d, wv_d, wo4_d, bo_d, out_d):
    nc = tc.nc
    persist = ctx.enter_context(tc.tile_pool(name="persist", bufs=1))
    stage = ctx.enter_context(tc.tile_pool(name="stage", bufs=2))
    epool = ctx.enter_context(tc.tile_pool(name="epool", bufs=6))
    espool = ctx.enter_context(tc.tile_pool(name="espool", bufs=2))
    spool = ctx.enter_context(tc.tile_pool(name="spool", bufs=2, space="PSUM"))
    cpool = ctx.enter_context(tc.tile_pool(name="cpool", bufs=4, space="PSUM"))

    # ---- input DMAs: x and weights FIRST (the projections gate the whole
    # pipeline); the 8MB mask after, so it doesn't hog the DMA queues ------
    xT = [stage.tile([128, N], bf16, tag="stage", name=f"xT{c}") for c in range(2)]
    for c in range(2):
        nc.sync.dma_start(xT[c][:], xT_d[c * 128:(c + 1) * 128, :])
    w_sb = {}
    for nm, dram in (("wq", wq_d), ("wk", wk_d), ("wv", wv_d)):
        w_sb[nm] = persist.tile([128, 2 * U], bf16, tag=nm, name=nm)
        for c in range(2):
            nc.sync.dma_start(w_sb[nm][:, c * U:(c + 1) * U],
                              dram[c * 128:(c + 1) * 128, :])
    wo_sb = persist.tile([128, 4 * U], bf16, tag="wo4")
    for t in range(4):
        nc.sync.dma_start(wo_sb[:, t * U:(t + 1) * U],
                          wo4_d[t * 128:(t + 1) * 128, :])
    bo_sb = persist.tile([1, U], f32, tag="bo")
    nc.sync.dma_start(bo_sb[:], bo_d.rearrange("(o n) -> o n", o=1))
    ones_f = persist.tile([1, 128], f32, tag="ones_f")
    nc.vector.memset(ones_f[:], 1.0)

    # ---- persistent SBUF tensors -------------------------------------------
    qT = [persist.tile([128, N], bf16, tag=f"qT{c}", name=f"qT{c}") for c in range(2)]
    kT = [persist.tile([128, N], bf16, tag=f"kT{c}", name=f"kT{c}") for c in range(2)]
    # augmented V: per (kb, h) a 64-col block [ones | V_h (32) | zeros31]
    v_aug = persist.tile([128, NB * H * 64], bf16, tag="vaug")
    nc.vector.memset(v_aug[:], 0.0)
    nc.vector.memset(
        v_aug.rearrange("p (b c) -> p b c", c=64)[:, :, 0:1], 1.0)
    # normalized context per pair tile t=2g+pi: rows 1-32 head 4g+pi,
    # rows 65-96 head 4g+pi+2 (rows 0/64 carry junk, matching wo4 zeros)
    ctxn = [persist.tile([128, N], bf16, tag=f"ctxn{t}", name=f"ctxn{t}")
            for t in range(4)]
    out_sb = persist.tile([128, NB * U], f32, tag="out_sb")
    # denominator scratch, one per pair tile
    zrec = [persist.tile([128, QW], f32, tag=f"zrec{t}", name=f"zrec{t}") for t in range(4)]
    zbs = [persist.tile([128, QW], f32, tag=f"zbs{t}", name=f"zbs{t}") for t in range(4)]
    zinv = [persist.tile([128, QW], f32, tag=f"zinv{t}", name=f"zinv{t}") for t in range(4)]
    for t in range(4):
        nc.vector.memset(zbs[t][:], 1.0)   # rows never broadcast stay 1.0

    # ---- projections (bf16 operands) ---------------------------------------
    for w, dst in (("wq", qT), ("wk", kT)):
        for g in range(2):
            for nn in range(2):
                ps = spool.tile([128, 2 * QW], f32, tag="s", name="projps")
                for half in range(2):
                    sl = slice(half * QW, (half + 1) * QW)
                    tok = slice(nn * 2 * QW + half * QW,
                                nn * 2 * QW + (half + 1) * QW)
                    for kc in range(2):
                        nc.tensor.matmul(
                            ps[:, sl],
                            w_sb[w][:, (kc * 2 + g) * 128:(kc * 2 + g + 1) * 128],
                            xT[kc][:, tok],
                            start=(kc == 0), stop=(kc == 1))
                nc.scalar.copy(dst[g][:, nn * 2 * QW:(nn + 1) * 2 * QW], ps[:])
    # V = x @ Wv -> strided into v_aug (head h of block kb at col 64h+1)
    for kb in range(NB):
        ps = spool.tile([128, U], f32, tag="s", name="vps")
        for kc in range(2):
            nc.tensor.matmul(
                ps[:],
                xT[kc][:, kb * 128:(kb + 1) * 128],
                w_sb["wv"][:, kc * U:(kc + 1) * U],
                start=(kc == 0), stop=(kc == 1))
        nc.vector.tensor_copy(
            v_aug.rearrange("p (b c) -> p b c", c=64)
            [:, kb * H:(kb + 1) * H, 1:1 + D],
            ps.rearrange("p (h d) -> p h d", d=D))

    # ---- adjacency mask (after compute inputs: big, not urgent) ------------
    m_sb = persist.tile([128, NB * N], bf16, tag="m")
    for kb in range(NB):
        nc.sync.dma_start(m_sb[:, kb * N:(kb + 1) * N],
                          adjT_d[kb * 128:(kb + 1) * 128, :])

    # ---- main attention loop, one-step PE software pipeline -----------------
    # pair p = 2g+pi covers heads 4g+pi, 4g+pi+2 (j = pi, pi+2)
    cps = {}

    def emit_scores(qc, kb):
        qs = qc * QW
        tiles = []
        for p in range(4):
            g, pi = p // 2, p % 2
            sps = spool.tile([128, 2 * QW], f32, tag="s", name=f"sps{qc}_{kb}_{p}")
            for jj in range(2):
                j = pi + 2 * jj
                nc.tensor.matmul(
                    sps[:, jj * QW:(jj + 1) * QW],
                    kT[g][32 * j:32 * (j + 1), kb * 128:(kb + 1) * 128],
                    qT[g][32 * j:32 * (j + 1), qs:qs + QW],
                    start=True, stop=True,
                    tile_position=(32 * j, 0))
            tiles.append(sps)
        return tiles

    def emit_expmask_pv(qc, kb, tiles):
        qs = qc * QW
        for p in range(4):
            g, pi = p // 2, p % 2
            sps = tiles[p]
            e = epool.tile([128, 2 * QW], bf16, tag="e", name=f"e{qc}_{kb}_{p}")
            me = m_sb[:, kb * N + qs:kb * N + qs + QW]
            if pair_mode(qc, kb, p) == "A":
                nc.scalar.activation(e[:], sps[:], EXP, scale=float(SCALE))
                nc.vector.tensor_tensor(
                    e.rearrange("p (j q) -> p j q", j=2),
                    e.rearrange("p (j q) -> p j q", j=2),
                    me.unsqueeze(1).broadcast_to([128, 2, QW]), MULT)
            else:  # "S"
                es = espool.tile([128, 2 * QW], i16, tag="es",
                                 name=f"es{qc}_{kb}_{p}")
                nc.vector.tensor_scalar(es[:], sps[:], float(SCH_A),
                                        float(SCH_B), MULT, ADD)
                nc.vector.tensor_tensor(
                    e.rearrange("p (j q) -> p j q", j=2),
                    es.bitcast(bf16).rearrange("p (j q) -> p j q", j=2),
                    me.unsqueeze(1).broadcast_to([128, 2, QW]), MULT)
            for jj in range(2):
                h = 4 * g + pi + 2 * jj
                ej = e[:, jj * QW:(jj + 1) * QW]
                nc.tensor.matmul(
                    cps[qc, p][64 * jj:64 * jj + 64, :],
                    v_aug[:, (kb * H + h) * 64:(kb * H + h + 1) * 64],
                    ej, start=(kb == 0), stop=(kb == NB - 1),
                    tile_position=(0, 64 * jj))

    def emit_finish_qc(qc):
        qs = qc * QW
        for t in range(4):
            nc.vector.tensor_copy(zrec[t][:], cps[qc, t][:])
            for jj in range(2):
                nc.sync.dma_start(
                    zbs[t][64 * jj + 1:64 * jj + 33, :],
                    zrec[t][64 * jj:64 * jj + 1, :]
                    .unsqueeze(1).broadcast_to([1, 32, QW]))
            nc.vector.reciprocal_approx_fast(zinv[t][:], zbs[t][:])
            nc.vector.tensor_tensor(ctxn[t][:, qs:qs + QW], cps[qc, t][:],
                                    zinv[t][:], MULT)
        for qb in range(qc * QC, (qc + 1) * QC):
            ops = spool.tile([128, U], f32, tag="s", name=f"ops{qb}")
            for t in range(4):
                nc.tensor.matmul(
                    ops[:],
                    ctxn[t][:, qb * 128:(qb + 1) * 128],
                    wo_sb[:, t * U:(t + 1) * U],
                    start=(t == 0), stop=False)
            nc.tensor.matmul(ops[:], ones_f[:], bo_sb[:],
                             start=False, stop=True, skip_group_check=True)
            nc.vector.tensor_copy(out_sb[:, qb * U:(qb + 1) * U], ops[:])
            nc.sync.dma_start(
                out_d[qb * 128:(qb + 1) * 128, :],
                out_sb[:, qb * U:(qb + 1) * U])

    prev = None
    prev_tiles = None
    for qc in range(QC):
        for p in range(4):
            cps[qc, p] = cpool.tile([128, QW], f32, tag="c", name=f"cps{qc}_{p}")
        for kb in range(NB):
            tiles = emit_scores(qc, kb)
            if prev is not None:
                emit_expmask_pv(prev[0], prev[1], prev_tiles)
                if prev[1] == NB - 1:
                    emit_finish_qc(prev[0])
            prev, prev_tiles = (qc, kb), tiles
    emit_expmask_pv(prev[0], prev[1], prev_tiles)
    emit_finish_qc(prev[0])


_CACHED = None


def _get_program():
    global _CACHED
    if _CACHED is None:
        _CACHED = build_program()
    return _CACHED


def _bf16(a):
    return np.asarray(a, dtype=ml_dtypes.bfloat16)


def _build_wo4(Wo):
    """Permute Wo rows into 4 chunks matching the fused ctx layout.

    Pair tile t = 2g+pi: row r=1..32 -> Wo row for head 4g+pi dim r-1;
    row r=65..96 -> head 4g+pi+2 dim r-65; other rows zero.
    """
    wo4 = np.zeros((4 * 128, U), dtype=np.float32)
    Wo = np.asarray(Wo, np.float32)
    for g in range(2):
        for pi in range(2):
            t = 2 * g + pi
            for jj in range(2):
                h = 4 * g + pi + 2 * jj
                wo4[t * 128 + 64 * jj + 1:t * 128 + 64 * jj + 33, :] = \
                    Wo[h * D:(h + 1) * D, :]
    return _bf16(wo4)


def kernel(node_features, adjacency_matrix, Wq, Wk, Wv, Wo, bo, **run_kwargs):
    nc = _get_program()
    xT = _bf16(np.transpose(np.asarray(node_features, np.float32), (0, 2, 1)))
    adjT = _bf16(np.transpose(np.asarray(adjacency_matrix), (0, 2, 1)))
    wo4 = _build_wo4(Wo)
    wq, wk, wv = _bf16(Wq), _bf16(Wk), _bf16(Wv)
    bo32 = np.asarray(bo, np.float32)
    in_maps = []
    for b in range(B):
        in_maps.append({
            "xT": np.ascontiguousarray(xT[b]),
            "adjT": np.ascontiguousarray(adjT[b]),
            "Wq": wq, "Wk": wk, "Wv": wv, "wo4": wo4,
            "bo": bo32,
        })
    res = run_bass_kernel_spmd(nc, in_maps, core_ids=list(range(B)), **run_kwargs)
    out = np.stack([res.results[b]["out"] for b in range(B)], axis=0)
    kernel.last_results = res
    return out


# revision 17
# speedup vs baseline: 1.1642x; 1.1642x over previous
"""MultiHeadGraphAttention Trainium2 kernel (pipelined v4).

Data-parallel over batch: core b computes batch element b (B=8, 8 cores).

Per-core math (one batch element, N=2048 nodes, U=256 units, H=8 heads, d=32):
  Q = x Wq, K = x Wk, V = x Wv
  sT[k,q]  = sum_d KT[d,k] QT[d,q]           (scores, transposed layout)
  e        = exp(sT/sqrt(d)) * adjT          (masked exp)
  ctxT[d,q] = sum_k V[k,d] e[k,q] ; Z[q] = sum_k e[k,q]
  out      = (ctxT/Z).T @ Wo + bo

Key structure:
  - (qc, kb) blocks of [128 keys x 512 queries]; per block 4 head-PAIR
    PSUM score tiles [128, 2x512] (2 banks, double-buffered) pipeline
    ACT exp / DVE mask / PE matmuls across pairs and iterations.
  - Pair (g, pi) covers heads 4g+pi and 4g+pi+2 so their fused PV+Z
    matmuls land at col positions 0 and 64 of one cps tile.
  - Z is FUSED into the PV matmul: stationary [ones | V_h | zeros31]
    (M=64) makes row 0 of each 64-row block the softmax denominator --
    no separate Z matmuls (eliminates 512 PE instructions).
  - Per-pair exp mode: 'A' = ACT exp + DVE mask multiply; 'S' = DVE-only
    Schraudolph (i16 = A*s + B from PSUM, bitcast to bf16 ~ exp) + mask.
    A rotating half-share of 'S' offloads ACT, the global bottleneck.
  - Host pre-transposes x/adj, converts everything to bf16, and permutes
    Wo into per-pair-tile chunks (wo4) matching the fused ctx layout.
"""

import sys

for p in ("/opt/trn_rl_repo",):
    if p not in sys.path:
        sys.path.insert(0, p)

from contextlib import ExitStack

import numpy as np
import ml_dtypes

import concourse.bass as bass
import concourse.mybir as mybir
import concourse.tile as tile
from concourse import bacc
from concourse.bass_utils import run_bass_kernel_spmd

B, N, U, H, D = 8, 2048, 256, 8, 32
NB = N // 128          # 16 key blocks of 128
QC = 4                 # q chunks
QW = N // QC           # 512 q per chunk
SCALE = 1.0 / np.sqrt(np.float32(D))
# Schraudolph bf16: bits(bf16(2^x)) ~= round(128*(x+127-0.04368))
SCH_A = 128.0 * float(SCALE) * 1.4426950408889634
SCH_B = (127.0 - 0.04368) * 128.0

f32 = mybir.dt.float32
bf16 = mybir.dt.bfloat16
i16 = mybir.dt.int16
EXP = mybir.ActivationFunctionType.Exp
MULT = mybir.AluOpType.mult
ADD = mybir.AluOpType.add


def pair_mode(qc, kb, p):
    """Engine path for masked-exp of head pair p of (qc,kb).

    Every second (qc,kb) sends one pair (rotating) to the DVE-Schraudolph
    path; the rest use ACT-exp + DVE-mask.
    """
    return "S" if (kb % 2 == 0 and p == (kb // 2) % 4) else "A"


def build_program():
    nc = bacc.Bacc("TRN2", target_bir_lowering=False, debug=False,
                   enable_asserts=False, num_devices=B)

    xT_d = nc.dram_tensor("xT", [U, N], bf16, kind="ExternalInput").ap()
    adjT_d = nc.dram_tensor("adjT", [N, N], bf16, kind="ExternalInput").ap()
    wq_d = nc.dram_tensor("Wq", [U, U], bf16, kind="ExternalInput").ap()
    wk_d = nc.dram_tensor("Wk", [U, U], bf16, kind="ExternalInput").ap()
    wv_d = nc.dram_tensor("Wv", [U, U], bf16, kind="ExternalInput").ap()
    wo4_d = nc.dram_tensor("wo4", [4 * 128, U], bf16, kind="ExternalInput").ap()
    bo_d = nc.dram_tensor("bo", [U], f32, kind="ExternalInput").ap()
    out_d = nc.dram_tensor("out", [N, U], f32, kind="ExternalOutput").ap()

    with tile.TileContext(nc) as tc:
        with ExitStack() as ctx:
            kernel_body(ctx, tc, xT_d, adjT_d, wq_d, wk_d, wv_d, wo4_d,
                        bo_d, out_d)
    nc.compile()
    return nc


def kernel_body(ctx, tc, xT_d, adjT_d, wq_d, wk_d, wv_d, wo4_d, bo_d, out_d):
    nc = tc.nc
    persist = ctx.enter_context(tc.tile_pool(name="persist", bufs=1))
    stage = ctx.enter_context(tc.tile_pool(name="stage", bufs=2))
    epool = ctx.enter_context(tc.tile_pool(name="epool", bufs=6))
    espool = ctx.enter_context(tc.tile_pool(name="espool", bufs=2))
    spool = ctx.enter_context(tc.tile_pool(name="spool", bufs=2, space="PSUM"))
    cpool = ctx.enter_context(tc.tile_pool(name="cpool", bufs=4, space="PSUM"))

    # ---- input DMAs: x and weights FIRST (the projections gate the whole
    # pipeline); the 8MB mask after, so it doesn't hog the DMA queues ------
    xT = [stage.tile([128, N], bf16, tag="stage", name=f"xT{c}") for c in range(2)]
    for c in range(2):
        nc.sync.dma_start(xT[c][:], xT_d[c * 128:(c + 1) * 128, :])
    w_sb = {}
    for nm, dram in (("wq", wq_d), ("wk", wk_d), ("wv", wv_d)):
        w_sb[nm] = persist.tile([128, 2 * U], bf16, tag=nm, name=nm)
        for c in range(2):
            nc.sync.dma_start(w_sb[nm][:, c * U:(c + 1) * U],
                              dram[c * 128:(c + 1) * 128, :])
    wo_sb = persist.tile([128, 4 * U], bf16, tag="wo4")
    for t in range(4):
        nc.sync.dma_start(wo_sb[:, t * U:(t + 1) * U],
                          wo4_d[t * 128:(t + 1) * 128, :])
    bo_sb = persist.tile([1, U], f32, tag="bo")
    nc.sync.dma_start(bo_sb[:], bo_d.rearrange("(o n) -> o n", o=1))
    ones_f = persist.tile([1, 128], f32, tag="ones_f")
    nc.vector.memset(ones_f[:], 1.0)

    # ---- persistent SBUF tensors -------------------------------------------
    qT = [persist.tile([128, N], bf16, tag=f"qT{c}", name=f"qT{c}") for c in range(2)]
    kT = [persist.tile([128, N], bf16, tag=f"kT{c}", name=f"kT{c}") for c in range(2)]
    # augmented V: per (kb, h) a 64-col block [ones | V_h (32) | zeros31]
    v_aug = persist.tile([128, NB * H * 64], bf16, tag="vaug")
    nc.vector.memset(v_aug[:], 0.0)
    nc.vector.memset(
        v_aug.rearrange("p (b c) -> p b c", c=64)[:, :, 0:1], 1.0)
    # normalized context per pair tile t=2g+pi: rows 1-32 head 4g+pi,
    # rows 65-96 head 4g+pi+2 (rows 0/64 carry junk, matching wo4 zeros)
    ctxn = [persist.tile([128, N], bf16, tag=f"ctxn{t}", name=f"ctxn{t}")
            for t in range(4)]
    out_sb = persist.tile([128, NB * U], f32, tag="out_sb")
    # denominator scratch, one per pair tile
    zrec = [persist.tile([128, QW], f32, tag=f"zrec{t}", name=f"zrec{t}") for t in range(4)]
    zbs = [persist.tile([128, QW], f32, tag=f"zbs{t}", name=f"zbs{t}") for t in range(4)]
    zinv = [persist.tile([128, QW], f32, tag=f"zinv{t}", name=f"zinv{t}") for t in range(4)]
    for t in range(4):
        nc.vector.memset(zbs[t][:], 1.0)   # rows never broadcast stay 1.0

    # ---- projections (bf16 operands) ---------------------------------------
    for w, dst in (("wq", qT), ("wk", kT)):
        for g in range(2):
            for nn in range(2):
                ps = spool.tile([128, 2 * QW], f32, tag="s", name="projps")
                for half in range(2):
                    sl = slice(half * QW, (half + 1) * QW)
                    tok = slice(nn * 2 * QW + half * QW,
                                nn * 2 * QW + (half + 1) * QW)
                    for kc in range(2):
                        nc.tensor.matmul(
                            ps[:, sl],
                            w_sb[w][:, (kc * 2 + g) * 128:(kc * 2 + g + 1) * 128],
                            xT[kc][:, tok],
                            start=(kc == 0), stop=(kc == 1))
                nc.scalar.copy(dst[g][:, nn * 2 * QW:(nn + 1) * 2 * QW], ps[:])
    # V = x @ Wv -> strided into v_aug (head h of block kb at col 64h+1)
    for kb in range(NB):
        ps = spool.tile([128, U], f32, tag="s", name="vps")
        for kc in range(2):
            nc.tensor.matmul(
                ps[:],
                xT[kc][:, kb * 128:(kb + 1) * 128],
                w_sb["wv"][:, kc * U:(kc + 1) * U],
                start=(kc == 0), stop=(kc == 1))
        nc.vector.tensor_copy(
            v_aug.rearrange("p (b c) -> p b c", c=64)
            [:, kb * H:(kb + 1) * H, 1:1 + D],
            ps.rearrange("p (h d) -> p h d", d=D))

    # ---- adjacency mask (after compute inputs: big, not urgent) ------------
    m_sb = persist.tile([128, NB * N], bf16, tag="m")
    for kb in range(NB):
        nc.sync.dma_start(m_sb[:, kb * N:(kb + 1) * N],
                          adjT_d[kb * 128:(kb + 1) * 128, :])

    # ---- main attention loop, one-step PE software pipeline -----------------
    # pair p = 2g+pi covers heads 4g+pi, 4g+pi+2 (j = pi, pi+2)
    cps = {}

    def emit_scores_pair(qc, kb, p):
        qs = qc * QW
        g, pi = p // 2, p % 2
        sps = spool.tile([128, 2 * QW], f32, tag="s", name=f"sps{qc}_{kb}_{p}")
        for jj in range(2):
            j = pi + 2 * jj
            nc.tensor.matmul(
                sps[:, jj * QW:(jj + 1) * QW],
                kT[g][32 * j:32 * (j + 1), kb * 128:(kb + 1) * 128],
                qT[g][32 * j:32 * (j + 1), qs:qs + QW],
                start=True, stop=True,
                tile_position=(32 * j, 0))
        return sps

    def emit_expmask_pv_pair(qc, kb, p, sps):
        qs = qc * QW
        g, pi = p // 2, p % 2
        e = epool.tile([128, 2 * QW], bf16, tag="e", name=f"e{qc}_{kb}_{p}")
        me = m_sb[:, kb * N + qs:kb * N + qs + QW]
        if pair_mode(qc, kb, p) == "A":
            nc.scalar.activation(e[:], sps[:], EXP, scale=float(SCALE))
            nc.vector.tensor_tensor(
                e.rearrange("p (j q) -> p j q", j=2),
                e.rearrange("p (j q) -> p j q", j=2),
                me.unsqueeze(1).broadcast_to([128, 2, QW]), MULT)
        else:  # "S"
            es = espool.tile([128, 2 * QW], i16, tag="es",
                             name=f"es{qc}_{kb}_{p}")
            nc.vector.tensor_scalar(es[:], sps[:], float(SCH_A),
                                    float(SCH_B), MULT, ADD)
            nc.vector.tensor_tensor(
                e.rearrange("p (j q) -> p j q", j=2),
                es.bitcast(bf16).rearrange("p (j q) -> p j q", j=2),
                me.unsqueeze(1).broadcast_to([128, 2, QW]), MULT)
        for jj in range(2):
            h = 4 * g + pi + 2 * jj
            ej = e[:, jj * QW:(jj + 1) * QW]
            nc.tensor.matmul(
                cps[qc, p][64 * jj:64 * jj + 64, :],
                v_aug[:, (kb * H + h) * 64:(kb * H + h + 1) * 64],
                ej, start=(kb == 0), stop=(kb == NB - 1),
                tile_position=(0, 64 * jj))

    def emit_finish_qc(qc):
        qs = qc * QW
        for t in range(4):
            nc.vector.tensor_copy(zrec[t][:], cps[qc, t][:])
            for jj in range(2):
                nc.sync.dma_start(
                    zbs[t][64 * jj + 1:64 * jj + 33, :],
                    zrec[t][64 * jj:64 * jj + 1, :]
                    .unsqueeze(1).broadcast_to([1, 32, QW]))
            nc.vector.reciprocal_approx_fast(zinv[t][:], zbs[t][:])
            nc.vector.tensor_tensor(ctxn[t][:, qs:qs + QW], cps[qc, t][:],
                                    zinv[t][:], MULT)
        for qb in range(qc * QC, (qc + 1) * QC):
            ops = spool.tile([128, U], f32, tag="s", name=f"ops{qb}")
            for t in range(4):
                nc.tensor.matmul(
                    ops[:],
                    ctxn[t][:, qb * 128:(qb + 1) * 128],
                    wo_sb[:, t * U:(t + 1) * U],
                    start=(t == 0), stop=False)
            nc.tensor.matmul(ops[:], ones_f[:], bo_sb[:],
                             start=False, stop=True, skip_group_check=True)
            nc.vector.tensor_copy(out_sb[:, qb * U:(qb + 1) * U], ops[:])
            nc.sync.dma_start(
                out_d[qb * 128:(qb + 1) * 128, :],
                out_sb[:, qb * U:(qb + 1) * U])

    # per-PAIR one-step software pipeline: scores(pair i+1) emitted before
    # expmask+PV(pair i), so PE bursts stay short and evenly spaced (HAM
    # re-throttles after ~3.4us of PE idle; per-iteration batching caused
    # exactly that in v4 and ran the whole kernel at 1.2GHz)
    prev = None
    for qc in range(QC):
        for p in range(4):
            cps[qc, p] = cpool.tile([128, QW], f32, tag="c", name=f"cps{qc}_{p}")
        for kb in range(NB):
            for p in range(4):
                sps = emit_scores_pair(qc, kb, p)
                if prev is not None:
                    emit_expmask_pv_pair(*prev)
                    if prev[1] == NB - 1 and prev[2] == 3:
                        emit_finish_qc(prev[0])
                prev = (qc, kb, p, sps)
    emit_expmask_pv_pair(*prev)
    emit_finish_qc(prev[0])


_CACHED = None


def _get_program():
    global _CACHED
    if _CACHED is None:
        _CACHED = build_program()
    return _CACHED


def _bf16(a):
    return np.asarray(a, dtype=ml_dtypes.bfloat16)


def _build_wo4(Wo):
    """Permute Wo rows into 4 chunks matching the fused ctx layout.

    Pair tile t = 2g+pi: row r=1..32 -> Wo row for head 4g+pi dim r-1;
    row r=65..96 -> head 4g+pi+2 dim r-65; other rows zero.
    """
    wo4 = np.zeros((4 * 128, U), dtype=np.float32)
    Wo = np.asarray(Wo, np.float32)
    for g in range(2):
        for pi in range(2):
            t = 2 * g + pi
            for jj in range(2):
                h = 4 * g + pi + 2 * jj
                wo4[t * 128 + 64 * jj + 1:t * 128 + 64 * jj + 33, :] = \
                    Wo[h * D:(h + 1) * D, :]
    return _bf16(wo4)


def kernel(node_features, adjacency_matrix, Wq, Wk, Wv, Wo, bo, **run_kwargs):
    nc = _get_program()
    xT = _bf16(np.transpose(np.asarray(node_features, np.float32), (0, 2, 1)))
    adjT = _bf16(np.transpose(np.asarray(adjacency_matrix), (0, 2, 1)))
    wo4 = _build_wo4(Wo)
    wq, wk, wv = _bf16(Wq), _bf16(Wk), _bf16(Wv)
    bo32 = np.asarray(bo, np.float32)
    in_maps = []
    for b in range(B):
        in_maps.append({
            "xT": np.ascontiguousarray(xT[b]),
            "adjT": np.ascontiguousarray(adjT[b]),
            "Wq": wq, "Wk": wk, "Wv": wv, "wo4": wo4,
            "bo": bo32,
        })
    res = run_bass_kernel_spmd(nc, in_maps, core_ids=list(range(B)), **run_kwargs)
    out = np.stack([res.results[b]["out"] for b in range(B)], axis=0)
    kernel.last_results = res
    return out


# revision 18
# speedup vs baseline: 1.1649x; 1.0006x over previous
"""MultiHeadGraphAttention Trainium2 kernel (pipelined v4).

Data-parallel over batch: core b computes batch element b (B=8, 8 cores).

Per-core math (one batch element, N=2048 nodes, U=256 units, H=8 heads, d=32):
  Q = x Wq, K = x Wk, V = x Wv
  sT[k,q]  = sum_d KT[d,k] QT[d,q]           (scores, transposed layout)
  e        = exp(sT/sqrt(d)) * adjT          (masked exp)
  ctxT[d,q] = sum_k V[k,d] e[k,q] ; Z[q] = sum_k e[k,q]
  out      = (ctxT/Z).T @ Wo + bo

Key structure:
  - (qc, kb) blocks of [128 keys x 512 queries]; per block 4 head-PAIR
    PSUM score tiles [128, 2x512] (2 banks, double-buffered) pipeline
    ACT exp / DVE mask / PE matmuls across pairs and iterations.
  - Pair (g, pi) covers heads 4g+pi and 4g+pi+2 so their fused PV+Z
    matmuls land at col positions 0 and 64 of one cps tile.
  - Z is FUSED into the PV matmul: stationary [ones | V_h | zeros31]
    (M=64) makes row 0 of each 64-row block the softmax denominator --
    no separate Z matmuls (eliminates 512 PE instructions).
  - Per-pair exp mode: 'A' = ACT exp + DVE mask multiply; 'S' = DVE-only
    Schraudolph (i16 = A*s + B from PSUM, bitcast to bf16 ~ exp) + mask.
    A rotating half-share of 'S' offloads ACT, the global bottleneck.
  - Host pre-transposes x/adj, converts everything to bf16, and permutes
    Wo into per-pair-tile chunks (wo4) matching the fused ctx layout.
"""

import sys

for p in ("/opt/trn_rl_repo",):
    if p not in sys.path:
        sys.path.insert(0, p)

from contextlib import ExitStack

import numpy as np
import ml_dtypes

import concourse.bass as bass
import concourse.mybir as mybir
import concourse.tile as tile
from concourse import bacc
from concourse.bass_utils import run_bass_kernel_spmd

B, N, U, H, D = 8, 2048, 256, 8, 32
NB = N // 128          # 16 key blocks of 128
QC = 4                 # q chunks
QW = N // QC           # 512 q per chunk
SCALE = 1.0 / np.sqrt(np.float32(D))
# Schraudolph bf16: bits(bf16(2^x)) ~= round(128*(x+127-0.04368))
SCH_A = 128.0 * float(SCALE) * 1.4426950408889634
SCH_B = (127.0 - 0.04368) * 128.0

f32 = mybir.dt.float32
bf16 = mybir.dt.bfloat16
i16 = mybir.dt.int16
EXP = mybir.ActivationFunctionType.Exp
MULT = mybir.AluOpType.mult
ADD = mybir.AluOpType.add


def pair_mode(qc, kb, p):
    """Engine path for masked-exp of head pair p of (qc,kb).

    Every second (qc,kb) sends one pair (rotating) to the DVE-Schraudolph
    path; the rest use ACT-exp + DVE-mask.
    """
    return "S" if (kb % 2 == 0 and p == (kb // 2) % 4) else "A"


def build_program():
    nc = bacc.Bacc("TRN2", target_bir_lowering=False, debug=False,
                   enable_asserts=False, num_devices=B)

    xT_d = nc.dram_tensor("xT", [U, N], bf16, kind="ExternalInput").ap()
    adjT_d = nc.dram_tensor("adjT", [N, N], bf16, kind="ExternalInput").ap()
    wq_d = nc.dram_tensor("Wq", [U, U], bf16, kind="ExternalInput").ap()
    wk_d = nc.dram_tensor("Wk", [U, U], bf16, kind="ExternalInput").ap()
    wv_d = nc.dram_tensor("Wv", [U, U], bf16, kind="ExternalInput").ap()
    wo4_d = nc.dram_tensor("wo4", [4 * 128, U], bf16, kind="ExternalInput").ap()
    bo_d = nc.dram_tensor("bo", [U], f32, kind="ExternalInput").ap()
    out_d = nc.dram_tensor("out", [N, U], f32, kind="ExternalOutput").ap()

    with tile.TileContext(nc) as tc:
        with ExitStack() as ctx:
            kernel_body(ctx, tc, xT_d, adjT_d, wq_d, wk_d, wv_d, wo4_d,
                        bo_d, out_d)
    nc.compile()
    return nc


def kernel_body(ctx, tc, xT_d, adjT_d, wq_d, wk_d, wv_d, wo4_d, bo_d, out_d):
    nc = tc.nc
    persist = ctx.enter_context(tc.tile_pool(name="persist", bufs=1))
    stage = ctx.enter_context(tc.tile_pool(name="stage", bufs=2))
    epool = ctx.enter_context(tc.tile_pool(name="epool", bufs=6))
    espool = ctx.enter_context(tc.tile_pool(name="espool", bufs=2))
    spool = ctx.enter_context(tc.tile_pool(name="spool", bufs=2, space="PSUM"))
    cpool = ctx.enter_context(tc.tile_pool(name="cpool", bufs=4, space="PSUM"))

    # ---- input DMAs: x and weights FIRST (the projections gate the whole
    # pipeline); the 8MB mask after, so it doesn't hog the DMA queues ------
    xT = [stage.tile([128, N], bf16, tag="stage", name=f"xT{c}") for c in range(2)]
    for c in range(2):
        nc.sync.dma_start(xT[c][:], xT_d[c * 128:(c + 1) * 128, :])
    w_sb = {}
    for nm, dram in (("wq", wq_d), ("wk", wk_d), ("wv", wv_d)):
        w_sb[nm] = persist.tile([128, 2 * U], bf16, tag=nm, name=nm)
        for c in range(2):
            nc.sync.dma_start(w_sb[nm][:, c * U:(c + 1) * U],
                              dram[c * 128:(c + 1) * 128, :])
    wo_sb = persist.tile([128, 4 * U], bf16, tag="wo4")
    for t in range(4):
        nc.sync.dma_start(wo_sb[:, t * U:(t + 1) * U],
                          wo4_d[t * 128:(t + 1) * 128, :])
    bo_sb = persist.tile([1, U], f32, tag="bo")
    nc.sync.dma_start(bo_sb[:], bo_d.rearrange("(o n) -> o n", o=1))
    ones_f = persist.tile([1, 128], f32, tag="ones_f")
    nc.vector.memset(ones_f[:], 1.0)

    # ---- persistent SBUF tensors -------------------------------------------
    qT = [persist.tile([128, N], bf16, tag=f"qT{c}", name=f"qT{c}") for c in range(2)]
    kT = [persist.tile([128, N], bf16, tag=f"kT{c}", name=f"kT{c}") for c in range(2)]
    # augmented V: per (kb, h) a 64-col block [ones | V_h (32) | zeros31]
    v_aug = persist.tile([128, NB * H * 64], bf16, tag="vaug")
    nc.vector.memset(v_aug[:], 0.0)
    nc.vector.memset(
        v_aug.rearrange("p (b c) -> p b c", c=64)[:, :, 0:1], 1.0)
    # normalized context per pair tile t=2g+pi: rows 1-32 head 4g+pi,
    # rows 65-96 head 4g+pi+2 (rows 0/64 carry junk, matching wo4 zeros)
    ctxn = [persist.tile([128, N], bf16, tag=f"ctxn{t}", name=f"ctxn{t}")
            for t in range(4)]
    out_sb = persist.tile([128, NB * U], f32, tag="out_sb")
    # denominator scratch, one per pair tile
    zrec = [persist.tile([128, QW], f32, tag=f"zrec{t}", name=f"zrec{t}") for t in range(4)]
    zbs = [persist.tile([128, QW], f32, tag=f"zbs{t}", name=f"zbs{t}") for t in range(4)]
    zinv = [persist.tile([128, QW], f32, tag=f"zinv{t}", name=f"zinv{t}") for t in range(4)]
    for t in range(4):
        nc.vector.memset(zbs[t][:], 1.0)   # rows never broadcast stay 1.0

    # ---- projections (bf16 operands) ---------------------------------------
    for w, dst in (("wq", qT), ("wk", kT)):
        for g in range(2):
            for nn in range(2):
                ps = spool.tile([128, 2 * QW], f32, tag="s", name="projps")
                for half in range(2):
                    sl = slice(half * QW, (half + 1) * QW)
                    tok = slice(nn * 2 * QW + half * QW,
                                nn * 2 * QW + (half + 1) * QW)
                    for kc in range(2):
                        nc.tensor.matmul(
                            ps[:, sl],
                            w_sb[w][:, (kc * 2 + g) * 128:(kc * 2 + g + 1) * 128],
                            xT[kc][:, tok],
                            start=(kc == 0), stop=(kc == 1))
                nc.scalar.copy(dst[g][:, nn * 2 * QW:(nn + 1) * 2 * QW], ps[:])
    # V = x @ Wv -> strided into v_aug (head h of block kb at col 64h+1)
    for kb in range(NB):
        ps = spool.tile([128, U], f32, tag="s", name="vps")
        for kc in range(2):
            nc.tensor.matmul(
                ps[:],
                xT[kc][:, kb * 128:(kb + 1) * 128],
                w_sb["wv"][:, kc * U:(kc + 1) * U],
                start=(kc == 0), stop=(kc == 1))
        nc.vector.tensor_copy(
            v_aug.rearrange("p (b c) -> p b c", c=64)
            [:, kb * H:(kb + 1) * H, 1:1 + D],
            ps.rearrange("p (h d) -> p h d", d=D))

    # ---- adjacency mask (after compute inputs: big, not urgent) ------------
    m_sb = persist.tile([128, NB * N], bf16, tag="m")
    for kb in range(NB):
        nc.sync.dma_start(m_sb[:, kb * N:(kb + 1) * N],
                          adjT_d[kb * 128:(kb + 1) * 128, :])

    # ---- main attention loop, one-step PE software pipeline -----------------
    # pair p = 2g+pi covers heads 4g+pi, 4g+pi+2 (j = pi, pi+2)
    cps = {}

    def emit_scores_pair(qc, kb, p):
        qs = qc * QW
        g, pi = p // 2, p % 2
        sps = spool.tile([128, 2 * QW], f32, tag="s", name=f"sps{qc}_{kb}_{p}")
        for jj in range(2):
            j = pi + 2 * jj
            nc.tensor.matmul(
                sps[:, jj * QW:(jj + 1) * QW],
                kT[g][32 * j:32 * (j + 1), kb * 128:(kb + 1) * 128],
                qT[g][32 * j:32 * (j + 1), qs:qs + QW],
                start=True, stop=True,
                tile_position=(32 * j, 0))
        return sps

    def emit_expmask_pv_pair(qc, kb, p, sps):
        qs = qc * QW
        g, pi = p // 2, p % 2
        e = epool.tile([128, 2 * QW], bf16, tag="e", name=f"e{qc}_{kb}_{p}")
        me = m_sb[:, kb * N + qs:kb * N + qs + QW]
        if pair_mode(qc, kb, p) == "A":
            nc.scalar.activation(e[:], sps[:], EXP, scale=float(SCALE))
            nc.vector.tensor_tensor(
                e.rearrange("p (j q) -> p j q", j=2),
                e.rearrange("p (j q) -> p j q", j=2),
                me.unsqueeze(1).broadcast_to([128, 2, QW]), MULT)
        else:  # "S"
            es = espool.tile([128, 2 * QW], i16, tag="es",
                             name=f"es{qc}_{kb}_{p}")
            nc.vector.tensor_scalar(es[:], sps[:], float(SCH_A),
                                    float(SCH_B), MULT, ADD)
            nc.vector.tensor_tensor(
                e.rearrange("p (j q) -> p j q", j=2),
                es.bitcast(bf16).rearrange("p (j q) -> p j q", j=2),
                me.unsqueeze(1).broadcast_to([128, 2, QW]), MULT)
        for jj in range(2):
            h = 4 * g + pi + 2 * jj
            ej = e[:, jj * QW:(jj + 1) * QW]
            nc.tensor.matmul(
                cps[qc, p][64 * jj:64 * jj + 64, :],
                v_aug[:, (kb * H + h) * 64:(kb * H + h + 1) * 64],
                ej, start=(kb == 0), stop=(kb == NB - 1),
                tile_position=(0, 64 * jj))

    def emit_finish_qc(qc):
        qs = qc * QW
        for t in range(4):
            nc.vector.tensor_copy(zrec[t][:], cps[qc, t][:])
            for jj in range(2):
                nc.sync.dma_start(
                    zbs[t][64 * jj + 1:64 * jj + 33, :],
                    zrec[t][64 * jj:64 * jj + 1, :]
                    .unsqueeze(1).broadcast_to([1, 32, QW]))
            nc.vector.reciprocal_approx_fast(zinv[t][:], zbs[t][:])
            nc.vector.tensor_tensor(ctxn[t][:, qs:qs + QW], cps[qc, t][:],
                                    zinv[t][:], MULT)
        for qb in range(qc * QC, (qc + 1) * QC):
            ops = spool.tile([128, U], f32, tag="s", name=f"ops{qb}")
            for t in range(4):
                nc.tensor.matmul(
                    ops[:],
                    ctxn[t][:, qb * 128:(qb + 1) * 128],
                    wo_sb[:, t * U:(t + 1) * U],
                    start=(t == 0), stop=False)
            nc.tensor.matmul(ops[:], ones_f[:], bo_sb[:],
                             start=False, stop=True, skip_group_check=True)
            nc.vector.tensor_copy(out_sb[:, qb * U:(qb + 1) * U], ops[:])
            nc.sync.dma_start(
                out_d[qb * 128:(qb + 1) * 128, :],
                out_sb[:, qb * U:(qb + 1) * U])

    # TWO-PAIR-group software pipeline: scores for pairs (2t, 2t+1) are
    # emitted back-to-back (their 4 row-groups run concurrently in the PE
    # array), then exp/mask/PV for the previous group.  Finer interleave
    # (v5) broke matmul concurrency (score row-tiles conflict with PV
    # col-tiles); coarser per-iteration batching (v4) left >3.4us PE idle
    # gaps, HAM-throttling the whole run to 1.2GHz.
    prev = None
    for qc in range(QC):
        for p in range(4):
            cps[qc, p] = cpool.tile([128, QW], f32, tag="c", name=f"cps{qc}_{p}")
        for kb in range(NB):
            for t in range(2):
                sp0 = emit_scores_pair(qc, kb, 2 * t)
                sp1 = emit_scores_pair(qc, kb, 2 * t + 1)
                if prev is not None:
                    (pqc, pkb, pt, psp0, psp1) = prev
                    emit_expmask_pv_pair(pqc, pkb, 2 * pt, psp0)
                    emit_expmask_pv_pair(pqc, pkb, 2 * pt + 1, psp1)
                    if pkb == NB - 1 and pt == 1:
                        emit_finish_qc(pqc)
                prev = (qc, kb, t, sp0, sp1)
    (pqc, pkb, pt, psp0, psp1) = prev
    emit_expmask_pv_pair(pqc, pkb, 2 * pt, psp0)
    emit_expmask_pv_pair(pqc, pkb, 2 * pt + 1, psp1)
    emit_finish_qc(pqc)


_CACHED = None


def _get_program():
    global _CACHED
    if _CACHED is None:
        _CACHED = build_program()
    return _CACHED


def _bf16(a):
    return np.asarray(a, dtype=ml_dtypes.bfloat16)


def _build_wo4(Wo):
    """Permute Wo rows into 4 chunks matching the fused ctx layout.

    Pair tile t = 2g+pi: row r=1..32 -> Wo row for head 4g+pi dim r-1;
    row r=65..96 -> head 4g+pi+2 dim r-65; other rows zero.
    """
    wo4 = np.zeros((4 * 128, U), dtype=np.float32)
    Wo = np.asarray(Wo, np.float32)
    for g in range(2):
        for pi in range(2):
            t = 2 * g + pi
            for jj in range(2):
                h = 4 * g + pi + 2 * jj
                wo4[t * 128 + 64 * jj + 1:t * 128 + 64 * jj + 33, :] = \
                    Wo[h * D:(h + 1) * D, :]
    return _bf16(wo4)


def kernel(node_features, adjacency_matrix, Wq, Wk, Wv, Wo, bo, **run_kwargs):
    nc = _get_program()
    xT = _bf16(np.transpose(np.asarray(node_features, np.float32), (0, 2, 1)))
    adjT = _bf16(np.transpose(np.asarray(adjacency_matrix), (0, 2, 1)))
    wo4 = _build_wo4(Wo)
    wq, wk, wv = _bf16(Wq), _bf16(Wk), _bf16(Wv)
    bo32 = np.asarray(bo, np.float32)
    in_maps = []
    for b in range(B):
        in_maps.append({
            "xT": np.ascontiguousarray(xT[b]),
            "adjT": np.ascontiguousarray(adjT[b]),
            "Wq": wq, "Wk": wk, "Wv": wv, "wo4": wo4,
            "bo": bo32,
        })
    res = run_bass_kernel_spmd(nc, in_maps, core_ids=list(range(B)), **run_kwargs)
    out = np.stack([res.results[b]["out"] for b in range(B)], axis=0)
    kernel.last_results = res
    return out


# revision 22
# speedup vs baseline: 1.1790x; 1.0121x over previous
"""MultiHeadGraphAttention Trainium2 kernel (pipelined v4).

Data-parallel over batch: core b computes batch element b (B=8, 8 cores).

Per-core math (one batch element, N=2048 nodes, U=256 units, H=8 heads, d=32):
  Q = x Wq, K = x Wk, V = x Wv
  sT[k,q]  = sum_d KT[d,k] QT[d,q]           (scores, transposed layout)
  e        = exp(sT/sqrt(d)) * adjT          (masked exp)
  ctxT[d,q] = sum_k V[k,d] e[k,q] ; Z[q] = sum_k e[k,q]
  out      = (ctxT/Z).T @ Wo + bo

Key structure:
  - (qc, kb) blocks of [128 keys x 512 queries]; per block 4 head-PAIR
    PSUM score tiles [128, 2x512] (2 banks, double-buffered) pipeline
    ACT exp / DVE mask / PE matmuls across pairs and iterations.
  - Pair (g, pi) covers heads 4g+pi and 4g+pi+2 so their fused PV+Z
    matmuls land at col positions 0 and 64 of one cps tile.
  - Z is FUSED into the PV matmul: stationary [ones | V_h | zeros31]
    (M=64) makes row 0 of each 64-row block the softmax denominator --
    no separate Z matmuls (eliminates 512 PE instructions).
  - Per-pair exp mode: 'A' = ACT exp + DVE mask multiply; 'S' = DVE-only
    Schraudolph (i16 = A*s + B from PSUM, bitcast to bf16 ~ exp) + mask.
    A rotating half-share of 'S' offloads ACT, the global bottleneck.
  - Host pre-transposes x/adj, converts everything to bf16, and permutes
    Wo into per-pair-tile chunks (wo4) matching the fused ctx layout.
"""

import sys

for p in ("/opt/trn_rl_repo",):
    if p not in sys.path:
        sys.path.insert(0, p)

from contextlib import ExitStack

import numpy as np
import ml_dtypes

import concourse.bass as bass
import concourse.mybir as mybir
import concourse.tile as tile
from concourse import bacc
from concourse.bass_utils import run_bass_kernel_spmd

B, N, U, H, D = 8, 2048, 256, 8, 32
NB = N // 128          # 16 key blocks of 128
QC = 4                 # q chunks
QW = N // QC           # 512 q per chunk
SCALE = 1.0 / np.sqrt(np.float32(D))
# Schraudolph bf16: bits(bf16(2^x)) ~= round(128*(x+127-0.04368))
SCH_A = 128.0 * float(SCALE) * 1.4426950408889634
SCH_B = (127.0 - 0.04368) * 128.0

f32 = mybir.dt.float32
bf16 = mybir.dt.bfloat16
i16 = mybir.dt.int16
EXP = mybir.ActivationFunctionType.Exp
MULT = mybir.AluOpType.mult
ADD = mybir.AluOpType.add


def pair_mode(qc, kb, p):
    """Engine path for masked-exp of head pair p of (qc,kb).

    Every second (qc,kb) sends one pair (rotating) to the DVE-Schraudolph
    path; the rest use ACT-exp + DVE-mask.
    """
    return "S" if (kb % 2 == 0 and p == (kb // 2) % 4) else "A"


def build_program():
    nc = bacc.Bacc("TRN2", target_bir_lowering=False, debug=False,
                   enable_asserts=False, num_devices=B)

    xT_d = nc.dram_tensor("xT", [U, N], bf16, kind="ExternalInput").ap()
    adjT_d = nc.dram_tensor("adjT", [N, N], bf16, kind="ExternalInput").ap()
    wq_d = nc.dram_tensor("Wq", [U, U], bf16, kind="ExternalInput").ap()
    wk_d = nc.dram_tensor("Wk", [U, U], bf16, kind="ExternalInput").ap()
    wv_d = nc.dram_tensor("Wv", [U, U], bf16, kind="ExternalInput").ap()
    wo4_d = nc.dram_tensor("wo4", [4 * 128, U], bf16, kind="ExternalInput").ap()
    bo_d = nc.dram_tensor("bo", [U], f32, kind="ExternalInput").ap()
    out_d = nc.dram_tensor("out", [N, U], f32, kind="ExternalOutput").ap()

    with tile.TileContext(nc) as tc:
        with ExitStack() as ctx:
            kernel_body(ctx, tc, xT_d, adjT_d, wq_d, wk_d, wv_d, wo4_d,
                        bo_d, out_d)
    nc.compile()
    return nc


def kernel_body(ctx, tc, xT_d, adjT_d, wq_d, wk_d, wv_d, wo4_d, bo_d, out_d):
    nc = tc.nc
    persist = ctx.enter_context(tc.tile_pool(name="persist", bufs=1))
    stage = ctx.enter_context(tc.tile_pool(name="stage", bufs=2))
    epool = ctx.enter_context(tc.tile_pool(name="epool", bufs=6))
    espool = ctx.enter_context(tc.tile_pool(name="espool", bufs=2))
    spool = ctx.enter_context(tc.tile_pool(name="spool", bufs=2, space="PSUM"))
    cpool = ctx.enter_context(tc.tile_pool(name="cpool", bufs=4, space="PSUM"))

    # ---- input DMAs: x and weights FIRST (the projections gate the whole
    # pipeline); the 8MB mask after, so it doesn't hog the DMA queues ------
    xT = [stage.tile([128, N], bf16, tag="stage", name=f"xT{c}") for c in range(2)]
    for c in range(2):
        nc.sync.dma_start(xT[c][:], xT_d[c * 128:(c + 1) * 128, :])
    w_sb = {}
    for nm, dram in (("wq", wq_d), ("wk", wk_d), ("wv", wv_d)):
        w_sb[nm] = persist.tile([128, 2 * U], bf16, tag=nm, name=nm)
        for c in range(2):
            nc.sync.dma_start(w_sb[nm][:, c * U:(c + 1) * U],
                              dram[c * 128:(c + 1) * 128, :])
    wo_sb = persist.tile([128, 4 * U], bf16, tag="wo4")
    for t in range(4):
        nc.sync.dma_start(wo_sb[:, t * U:(t + 1) * U],
                          wo4_d[t * 128:(t + 1) * 128, :])
    bo_sb = persist.tile([1, U], f32, tag="bo")
    nc.sync.dma_start(bo_sb[:], bo_d.rearrange("(o n) -> o n", o=1))
    ones_f = persist.tile([1, 128], f32, tag="ones_f")
    nc.vector.memset(ones_f[:], 1.0)

    # ---- persistent SBUF tensors -------------------------------------------
    qT = [persist.tile([128, N], bf16, tag=f"qT{c}", name=f"qT{c}") for c in range(2)]
    kT = [persist.tile([128, N], bf16, tag=f"kT{c}", name=f"kT{c}") for c in range(2)]
    # augmented V: per (kb, h) a 64-col block [ones | V_h (32) | zeros31]
    v_aug = persist.tile([128, NB * H * 64], bf16, tag="vaug")
    nc.vector.memset(v_aug[:], 0.0)
    nc.vector.memset(
        v_aug.rearrange("p (b c) -> p b c", c=64)[:, :, 0:1], 1.0)
    # normalized context per pair tile t=2g+pi: rows 1-32 head 4g+pi,
    # rows 65-96 head 4g+pi+2 (rows 0/64 carry junk, matching wo4 zeros)
    ctxn = [persist.tile([128, N], bf16, tag=f"ctxn{t}", name=f"ctxn{t}")
            for t in range(4)]
    out_sb = persist.tile([128, NB * U], f32, tag="out_sb")
    # denominator scratch; zbs/zinv are single wide tiles so one
    # reciprocal_approx_fast covers all four pair tiles per qc
    zrec = [persist.tile([128, QW], f32, tag=f"zrec{t}", name=f"zrec{t}") for t in range(4)]
    zbs4 = persist.tile([128, 4 * QW], f32, tag="zbs4")
    zinv4 = persist.tile([128, 4 * QW], f32, tag="zinv4")
    zbs = [zbs4[:, t * QW:(t + 1) * QW] for t in range(4)]
    zinv = [zinv4[:, t * QW:(t + 1) * QW] for t in range(4)]
    nc.vector.memset(zbs4[:], 1.0)   # rows never broadcast stay 1.0

    # ---- projections (bf16 operands) ---------------------------------------
    for w, dst in (("wq", qT), ("wk", kT)):
        for g in range(2):
            for nn in range(2):
                ps = spool.tile([128, 2 * QW], f32, tag="s", name="projps")
                for half in range(2):
                    sl = slice(half * QW, (half + 1) * QW)
                    tok = slice(nn * 2 * QW + half * QW,
                                nn * 2 * QW + (half + 1) * QW)
                    for kc in range(2):
                        nc.tensor.matmul(
                            ps[:, sl],
                            w_sb[w][:, (kc * 2 + g) * 128:(kc * 2 + g + 1) * 128],
                            xT[kc][:, tok],
                            start=(kc == 0), stop=(kc == 1))
                nc.scalar.copy(dst[g][:, nn * 2 * QW:(nn + 1) * 2 * QW], ps[:])
    # V = x @ Wv -> strided into v_aug (head h of block kb at col 64h+1)
    for kb in range(NB):
        ps = spool.tile([128, U], f32, tag="s", name="vps")
        for kc in range(2):
            nc.tensor.matmul(
                ps[:],
                xT[kc][:, kb * 128:(kb + 1) * 128],
                w_sb["wv"][:, kc * U:(kc + 1) * U],
                start=(kc == 0), stop=(kc == 1))
        nc.vector.tensor_copy(
            v_aug.rearrange("p (b c) -> p b c", c=64)
            [:, kb * H:(kb + 1) * H, 1:1 + D],
            ps.rearrange("p (h d) -> p h d", d=D))

    # ---- adjacency mask (after compute inputs: big, not urgent) ------------
    m_sb = persist.tile([128, NB * N], bf16, tag="m")
    for kb in range(NB):
        nc.sync.dma_start(m_sb[:, kb * N:(kb + 1) * N],
                          adjT_d[kb * 128:(kb + 1) * 128, :])

    # ---- main attention loop, one-step PE software pipeline -----------------
    # pair p = 2g+pi covers heads 4g+pi, 4g+pi+2 (j = pi, pi+2)
    cps = {}

    def emit_scores_pair(qc, kb, p):
        qs = qc * QW
        g, pi = p // 2, p % 2
        sps = spool.tile([128, 2 * QW], f32, tag="s", name=f"sps{qc}_{kb}_{p}")
        for jj in range(2):
            j = pi + 2 * jj
            nc.tensor.matmul(
                sps[:, jj * QW:(jj + 1) * QW],
                kT[g][32 * j:32 * (j + 1), kb * 128:(kb + 1) * 128],
                qT[g][32 * j:32 * (j + 1), qs:qs + QW],
                start=True, stop=True,
                tile_position=(32 * j, 0))
        return sps

    def emit_expmask_pv_pair(qc, kb, p, sps):
        qs = qc * QW
        g, pi = p // 2, p % 2
        e = epool.tile([128, 2 * QW], bf16, tag="e", name=f"e{qc}_{kb}_{p}")
        me = m_sb[:, kb * N + qs:kb * N + qs + QW]
        if pair_mode(qc, kb, p) == "A":
            nc.scalar.activation(e[:], sps[:], EXP, scale=float(SCALE))
            nc.vector.tensor_tensor(
                e.rearrange("p (j q) -> p j q", j=2),
                e.rearrange("p (j q) -> p j q", j=2),
                me.unsqueeze(1).broadcast_to([128, 2, QW]), MULT)
        else:  # "S"
            es = espool.tile([128, 2 * QW], i16, tag="es",
                             name=f"es{qc}_{kb}_{p}")
            nc.vector.tensor_scalar(es[:], sps[:], float(SCH_A),
                                    float(SCH_B), MULT, ADD)
            nc.vector.tensor_tensor(
                e.rearrange("p (j q) -> p j q", j=2),
                es.bitcast(bf16).rearrange("p (j q) -> p j q", j=2),
                me.unsqueeze(1).broadcast_to([128, 2, QW]), MULT)
        for jj in range(2):
            h = 4 * g + pi + 2 * jj
            ej = e[:, jj * QW:(jj + 1) * QW]
            nc.tensor.matmul(
                cps[qc, p][64 * jj:64 * jj + 64, :],
                v_aug[:, (kb * H + h) * 64:(kb * H + h + 1) * 64],
                ej, start=(kb == 0), stop=(kb == NB - 1),
                tile_position=(0, 64 * jj))

    def emit_normalize_qc(qc):
        qs = qc * QW
        for t in range(4):
            nc.vector.tensor_copy(zrec[t][:], cps[qc, t][:])
            for jj in range(2):
                nc.sync.dma_start(
                    zbs[t][64 * jj + 1:64 * jj + 33, :],
                    zrec[t][64 * jj:64 * jj + 1, :]
                    .unsqueeze(1).broadcast_to([1, 32, QW]))
        nc.vector.reciprocal_approx_fast(zinv4[:], zbs4[:])
        for t in range(4):
            nc.vector.tensor_tensor(ctxn[t][:, qs:qs + QW], cps[qc, t][:],
                                    zinv[t][:], MULT)

    def emit_outproj_qc(qc):
        for qb in range(qc * QC, (qc + 1) * QC):
            ops = spool.tile([128, U], f32, tag="s", name=f"ops{qb}")
            for t in range(4):
                nc.tensor.matmul(
                    ops[:],
                    ctxn[t][:, qb * 128:(qb + 1) * 128],
                    wo_sb[:, t * U:(t + 1) * U],
                    start=(t == 0), stop=False)
            nc.tensor.matmul(ops[:], ones_f[:], bo_sb[:],
                             start=False, stop=True, skip_group_check=True)
            nc.vector.tensor_copy(out_sb[:, qb * U:(qb + 1) * U], ops[:])
            nc.sync.dma_start(
                out_d[qb * 128:(qb + 1) * 128, :],
                out_sb[:, qb * U:(qb + 1) * U])

    # TWO-PAIR-group software pipeline: scores for pairs (2t, 2t+1) are
    # emitted back-to-back (their 4 row-groups run concurrently in the PE
    # array), then exp/mask/PV for the previous group.  Finer interleave
    # (v5) broke matmul concurrency (score row-tiles conflict with PV
    # col-tiles); coarser per-iteration batching (v4) left >3.4us PE idle
    # gaps, HAM-throttling the whole run to 1.2GHz.
    prev = None
    pending_outproj = []
    for qc in range(QC):
        for p in range(4):
            cps[qc, p] = cpool.tile([128, QW], f32, tag="c", name=f"cps{qc}_{p}")
        for kb in range(NB):
            for t in range(2):
                sp0 = emit_scores_pair(qc, kb, 2 * t)
                sp1 = emit_scores_pair(qc, kb, 2 * t + 1)
                if prev is not None:
                    (pqc, pkb, pt, psp0, psp1) = prev
                    emit_expmask_pv_pair(pqc, pkb, 2 * pt, psp0)
                    emit_expmask_pv_pair(pqc, pkb, 2 * pt + 1, psp1)
                    if pkb == NB - 1 and pt == 1:
                        # normalize NOW (frees cps for the new qc) ...
                        emit_normalize_qc(pqc)
                        pending_outproj.append(pqc)
                    elif pending_outproj and pkb == 2 and pt == 1:
                        # ... out-projection a few blocks later, once the
                        # new qc's score pipeline is in steady state
                        emit_outproj_qc(pending_outproj.pop(0))
                prev = (qc, kb, t, sp0, sp1)
    (pqc, pkb, pt, psp0, psp1) = prev
    emit_expmask_pv_pair(pqc, pkb, 2 * pt, psp0)
    emit_expmask_pv_pair(pqc, pkb, 2 * pt + 1, psp1)
    emit_normalize_qc(pqc)
    pending_outproj.append(pqc)
    for f in pending_outproj:
        emit_outproj_qc(f)


_CACHED = None


def _get_program():
    global _CACHED
    if _CACHED is None:
        _CACHED = build_program()
    return _CACHED


def _bf16(a):
    return np.asarray(a, dtype=ml_dtypes.bfloat16)


def _build_wo4(Wo):
    """Permute Wo rows into 4 chunks matching the fused ctx layout.

    Pair tile t = 2g+pi: row r=1..32 -> Wo row for head 4g+pi dim r-1;
    row r=65..96 -> head 4g+pi+2 dim r-65; other rows zero.
    """
    wo4 = np.zeros((4 * 128, U), dtype=np.float32)
    Wo = np.asarray(Wo, np.float32)
    for g in range(2):
        for pi in range(2):
            t = 2 * g + pi
            for jj in range(2):
                h = 4 * g + pi + 2 * jj
                wo4[t * 128 + 64 * jj + 1:t * 128 + 64 * jj + 33, :] = \
                    Wo[h * D:(h + 1) * D, :]
    return _bf16(wo4)


def kernel(node_features, adjacency_matrix, Wq, Wk, Wv, Wo, bo, **run_kwargs):
    nc = _get_program()
    xT = _bf16(np.transpose(np.asarray(node_features, np.float32), (0, 2, 1)))
    adjT = _bf16(np.transpose(np.asarray(adjacency_matrix), (0, 2, 1)))
    wo4 = _build_wo4(Wo)
    wq, wk, wv = _bf16(Wq), _bf16(Wk), _bf16(Wv)
    bo32 = np.asarray(bo, np.float32)
    in_maps = []
    for b in range(B):
        in_maps.append({
            "xT": np.ascontiguousarray(xT[b]),
            "adjT": np.ascontiguousarray(adjT[b]),
            "Wq": wq, "Wk": wk, "Wv": wv, "wo4": wo4,
            "bo": bo32,
        })
    res = run_bass_kernel_spmd(nc, in_maps, core_ids=list(range(B)), **run_kwargs)
    out = np.stack([res.results[b]["out"] for b in range(B)], axis=0)
    kernel.last_results = res
    return out


# revision 28
# speedup vs baseline: 1.2098x; 1.0261x over previous
"""MultiHeadGraphAttention Trainium2 kernel (pipelined v4).

Data-parallel over batch: core b computes batch element b (B=8, 8 cores).

Per-core math (one batch element, N=2048 nodes, U=256 units, H=8 heads, d=32):
  Q = x Wq, K = x Wk, V = x Wv
  sT[k,q]  = sum_d KT[d,k] QT[d,q]           (scores, transposed layout)
  e        = exp(sT/sqrt(d)) * adjT          (masked exp)
  ctxT[d,q] = sum_k V[k,d] e[k,q] ; Z[q] = sum_k e[k,q]
  out      = (ctxT/Z).T @ Wo + bo

Key structure:
  - (qc, kb) blocks of [128 keys x 512 queries]; per block 4 head-PAIR
    PSUM score tiles [128, 2x512] (2 banks, double-buffered) pipeline
    ACT exp / DVE mask / PE matmuls across pairs and iterations.
  - Pair (g, pi) covers heads 4g+pi and 4g+pi+2 so their fused PV+Z
    matmuls land at col positions 0 and 64 of one cps tile.
  - Z is FUSED into the PV matmul: stationary [ones | V_h | zeros31]
    (M=64) makes row 0 of each 64-row block the softmax denominator --
    no separate Z matmuls (eliminates 512 PE instructions).
  - Per-pair exp mode: 'A' = ACT exp + DVE mask multiply; 'S' = DVE-only
    Schraudolph (i16 = A*s + B from PSUM, bitcast to bf16 ~ exp) + mask.
    A rotating half-share of 'S' offloads ACT, the global bottleneck.
  - Host pre-transposes x/adj, converts everything to bf16, and permutes
    Wo into per-pair-tile chunks (wo4) matching the fused ctx layout.
"""

import sys

for p in ("/opt/trn_rl_repo",):
    if p not in sys.path:
        sys.path.insert(0, p)

from contextlib import ExitStack

import numpy as np
import ml_dtypes

import concourse.bass as bass
import concourse.mybir as mybir
import concourse.tile as tile
from concourse import bacc
from concourse.bass_utils import run_bass_kernel_spmd

B, N, U, H, D = 8, 2048, 256, 8, 32
NB = N // 128          # 16 key blocks of 128
QC = 4                 # q chunks
QW = N // QC           # 512 q per chunk
SCALE = 1.0 / np.sqrt(np.float32(D))
# Schraudolph bf16: bits(bf16(2^x)) ~= round(128*(x+127-0.04368))
SCH_A = 128.0 * float(SCALE) * 1.4426950408889634
SCH_B = (127.0 - 0.04368) * 128.0

f32 = mybir.dt.float32
bf16 = mybir.dt.bfloat16
i16 = mybir.dt.int16
EXP = mybir.ActivationFunctionType.Exp
MULT = mybir.AluOpType.mult
ADD = mybir.AluOpType.add


def pair_mode(qc, kb, p):
    """Engine path for masked-exp of head pair p of (qc,kb).

    Every second (qc,kb) sends one pair (rotating) to the DVE-Schraudolph
    path; the rest use ACT-exp + DVE-mask.
    """
    return "S" if (kb % 2 == 0 and p == (kb // 2) % 4) else "A"


def build_program():
    nc = bacc.Bacc("TRN2", target_bir_lowering=False, debug=False,
                   enable_asserts=False, num_devices=B)

    xT_d = nc.dram_tensor("xT", [U, N], bf16, kind="ExternalInput").ap()
    adjT_d = nc.dram_tensor("adjT", [N, N], bf16, kind="ExternalInput").ap()
    wq_d = nc.dram_tensor("Wq", [U, U], bf16, kind="ExternalInput").ap()
    wk_d = nc.dram_tensor("Wk", [U, U], bf16, kind="ExternalInput").ap()
    wv_d = nc.dram_tensor("Wv", [U, U], bf16, kind="ExternalInput").ap()
    wo4_d = nc.dram_tensor("wo4", [4 * 128, U], bf16, kind="ExternalInput").ap()
    bo_d = nc.dram_tensor("bo", [U], f32, kind="ExternalInput").ap()
    out_d = nc.dram_tensor("out", [N, U], f32, kind="ExternalOutput").ap()

    with tile.TileContext(nc) as tc:
        with ExitStack() as ctx:
            kernel_body(ctx, tc, xT_d, adjT_d, wq_d, wk_d, wv_d, wo4_d,
                        bo_d, out_d)
    nc.compile()
    return nc


def kernel_body(ctx, tc, xT_d, adjT_d, wq_d, wk_d, wv_d, wo4_d, bo_d, out_d):
    nc = tc.nc
    persist = ctx.enter_context(tc.tile_pool(name="persist", bufs=1))
    stage = ctx.enter_context(tc.tile_pool(name="stage", bufs=2))
    epool = ctx.enter_context(tc.tile_pool(name="epool", bufs=8))
    espool = ctx.enter_context(tc.tile_pool(name="espool", bufs=2))
    spool = ctx.enter_context(tc.tile_pool(name="spool", bufs=2, space="PSUM"))
    cpool = ctx.enter_context(tc.tile_pool(name="cpool", bufs=4, space="PSUM"))

    # ---- input DMAs: x and weights FIRST (the projections gate the whole
    # pipeline); the 8MB mask after, so it doesn't hog the DMA queues ------
    xT = [stage.tile([128, N], bf16, tag="stage", name=f"xT{c}") for c in range(2)]
    for c in range(2):
        nc.sync.dma_start(xT[c][:], xT_d[c * 128:(c + 1) * 128, :])
    w_sb = {}
    for nm, dram in (("wq", wq_d), ("wk", wk_d), ("wv", wv_d)):
        w_sb[nm] = persist.tile([128, 2 * U], bf16, tag=nm, name=nm)
        for c in range(2):
            nc.sync.dma_start(w_sb[nm][:, c * U:(c + 1) * U],
                              dram[c * 128:(c + 1) * 128, :])
    wo_sb = persist.tile([128, 4 * U], bf16, tag="wo4")
    for t in range(4):
        nc.sync.dma_start(wo_sb[:, t * U:(t + 1) * U],
                          wo4_d[t * 128:(t + 1) * 128, :])
    # bias broadcast to all 128 partitions once; added during the PSUM->SBUF
    # copy of the out-projection (cheaper than a PE bias matmul)
    bo_sb = persist.tile([1, U], f32, tag="bo")
    nc.sync.dma_start(bo_sb[:], bo_d.rearrange("(o n) -> o n", o=1))
    bo_bc = persist.tile([128, U], f32, tag="bo_bc")
    nc.sync.dma_start(bo_bc[:],
                      bo_sb[:].unsqueeze(1).broadcast_to([1, 128, U]))

    # ---- persistent SBUF tensors -------------------------------------------
    qT = [persist.tile([128, N], bf16, tag=f"qT{c}", name=f"qT{c}") for c in range(2)]
    kT = [persist.tile([128, N], bf16, tag=f"kT{c}", name=f"kT{c}") for c in range(2)]
    # augmented V: per (kb, h) a 64-col block [ones | V_h (32) | zeros31]
    v_aug = persist.tile([128, NB * H * 64], bf16, tag="vaug")
    nc.vector.memset(v_aug[:], 0.0)
    nc.vector.memset(
        v_aug.rearrange("p (b c) -> p b c", c=64)[:, :, 0:1], 1.0)
    # normalized context per pair tile t=2g+pi: rows 1-32 head 4g+pi,
    # rows 65-96 head 4g+pi+2 (rows 0/64 carry junk, matching wo4 zeros)
    ctxn = [persist.tile([128, N], bf16, tag=f"ctxn{t}", name=f"ctxn{t}")
            for t in range(4)]
    out_sb = persist.tile([128, NB * U], f32, tag="out_sb")
    # denominator scratch; zbs/zinv are single wide tiles so one
    # reciprocal_approx_fast covers all four pair tiles per qc
    zrec = [persist.tile([128, QW], f32, tag=f"zrec{t}", name=f"zrec{t}") for t in range(4)]
    zbs4 = persist.tile([128, 4 * QW], f32, tag="zbs4")
    zinv4 = persist.tile([128, 4 * QW], f32, tag="zinv4")
    zbs = [zbs4[:, t * QW:(t + 1) * QW] for t in range(4)]
    zinv = [zinv4[:, t * QW:(t + 1) * QW] for t in range(4)]
    nc.vector.memset(zbs4[:], 1.0)   # rows never broadcast stay 1.0

    # ---- projections (bf16 operands) ---------------------------------------
    for w, dst in (("wq", qT), ("wk", kT)):
        for g in range(2):
            for nn in range(2):
                ps = spool.tile([128, 2 * QW], f32, tag="s", name="projps")
                for half in range(2):
                    sl = slice(half * QW, (half + 1) * QW)
                    tok = slice(nn * 2 * QW + half * QW,
                                nn * 2 * QW + (half + 1) * QW)
                    for kc in range(2):
                        nc.tensor.matmul(
                            ps[:, sl],
                            w_sb[w][:, (kc * 2 + g) * 128:(kc * 2 + g + 1) * 128],
                            xT[kc][:, tok],
                            start=(kc == 0), stop=(kc == 1))
                if w == "wq":   # split prologue copies across ACT and DVE
                    nc.scalar.copy(dst[g][:, nn * 2 * QW:(nn + 1) * 2 * QW], ps[:])
                else:
                    nc.vector.tensor_copy(dst[g][:, nn * 2 * QW:(nn + 1) * 2 * QW], ps[:])
    # V = x @ Wv -> strided into v_aug (head h of block kb at col 64h+1)
    for kb in range(NB):
        ps = spool.tile([128, U], f32, tag="s", name="vps")
        for kc in range(2):
            nc.tensor.matmul(
                ps[:],
                xT[kc][:, kb * 128:(kb + 1) * 128],
                w_sb["wv"][:, kc * U:(kc + 1) * U],
                start=(kc == 0), stop=(kc == 1))
        nc.vector.tensor_copy(
            v_aug.rearrange("p (b c) -> p b c", c=64)
            [:, kb * H:(kb + 1) * H, 1:1 + D],
            ps.rearrange("p (h d) -> p h d", d=D))

    # ---- adjacency mask (after compute inputs: big, not urgent) ------------
    m_sb = persist.tile([128, NB * N], bf16, tag="m")
    for kb in range(NB):
        nc.sync.dma_start(m_sb[:, kb * N:(kb + 1) * N],
                          adjT_d[kb * 128:(kb + 1) * 128, :])

    # ---- main attention loop, one-step PE software pipeline -----------------
    # pair p = 2g+pi covers heads 4g+pi, 4g+pi+2 (j = pi, pi+2)
    cps = {}

    def emit_scores_pair(qc, kb, p):
        qs = qc * QW
        g, pi = p // 2, p % 2
        sps = spool.tile([128, 2 * QW], f32, tag="s", name=f"sps{qc}_{kb}_{p}")
        for jj in range(2):
            j = pi + 2 * jj
            nc.tensor.matmul(
                sps[:, jj * QW:(jj + 1) * QW],
                kT[g][32 * j:32 * (j + 1), kb * 128:(kb + 1) * 128],
                qT[g][32 * j:32 * (j + 1), qs:qs + QW],
                start=True, stop=True,
                tile_position=(32 * j, 0))
        return sps

    def emit_expmask_pv_pair(qc, kb, p, sps):
        qs = qc * QW
        g, pi = p // 2, p % 2
        e = epool.tile([128, 2 * QW], bf16, tag="e", name=f"e{qc}_{kb}_{p}")
        me = m_sb[:, kb * N + qs:kb * N + qs + QW]
        if pair_mode(qc, kb, p) == "A":
            nc.scalar.activation(e[:], sps[:], EXP, scale=float(SCALE))
            nc.vector.tensor_tensor(
                e.rearrange("p (j q) -> p j q", j=2),
                e.rearrange("p (j q) -> p j q", j=2),
                me.unsqueeze(1).broadcast_to([128, 2, QW]), MULT)
        else:  # "S"
            es = espool.tile([128, 2 * QW], i16, tag="es",
                             name=f"es{qc}_{kb}_{p}")
            nc.vector.tensor_scalar(es[:], sps[:], float(SCH_A),
                                    float(SCH_B), MULT, ADD)
            nc.vector.tensor_tensor(
                e.rearrange("p (j q) -> p j q", j=2),
                es.bitcast(bf16).rearrange("p (j q) -> p j q", j=2),
                me.unsqueeze(1).broadcast_to([128, 2, QW]), MULT)
        for jj in range(2):
            h = 4 * g + pi + 2 * jj
            ej = e[:, jj * QW:(jj + 1) * QW]
            nc.tensor.matmul(
                cps[qc, p][64 * jj:64 * jj + 64, :],
                v_aug[:, (kb * H + h) * 64:(kb * H + h + 1) * 64],
                ej, start=(kb == 0), stop=(kb == NB - 1),
                tile_position=(0, 64 * jj))

    def emit_normalize_qc(qc):
        qs = qc * QW
        for t in range(4):
            nc.vector.tensor_copy(zrec[t][:], cps[qc, t][:])
            for jj in range(2):
                nc.sync.dma_start(
                    zbs[t][64 * jj + 1:64 * jj + 33, :],
                    zrec[t][64 * jj:64 * jj + 1, :]
                    .unsqueeze(1).broadcast_to([1, 32, QW]))
        nc.vector.reciprocal_approx_fast(zinv4[:], zbs4[:])
        for t in range(4):
            nc.vector.tensor_tensor(ctxn[t][:, qs:qs + QW], cps[qc, t][:],
                                    zinv[t][:], MULT)

    def emit_outproj_qb(qb):
        ops = spool.tile([128, U], f32, tag="s", name=f"ops{qb}")
        for t in range(4):
            nc.tensor.matmul(
                ops[:],
                ctxn[t][:, qb * 128:(qb + 1) * 128],
                wo_sb[:, t * U:(t + 1) * U],
                start=(t == 0), stop=(t == 3))
        nc.vector.tensor_tensor(out_sb[:, qb * U:(qb + 1) * U], ops[:],
                                bo_bc[:], ADD)
        nc.sync.dma_start(
            out_d[qb * 128:(qb + 1) * 128, :],
            out_sb[:, qb * U:(qb + 1) * U])

    # TWO-PAIR-group software pipeline: scores for pairs (2t, 2t+1) are
    # emitted back-to-back (their 4 row-groups run concurrently in the PE
    # array), then exp/mask/PV for the previous group.  Finer interleave
    # (v5) broke matmul concurrency (score row-tiles conflict with PV
    # col-tiles); coarser per-iteration batching (v4) left >3.4us PE idle
    # gaps, HAM-throttling the whole run to 1.2GHz.
    prev = None
    pending_outproj = []
    for qc in range(QC):
        for p in range(4):
            cps[qc, p] = cpool.tile([128, QW], f32, tag="c", name=f"cps{qc}_{p}")
        for kb in range(NB):
            for t in range(2):
                sp0 = emit_scores_pair(qc, kb, 2 * t)
                sp1 = emit_scores_pair(qc, kb, 2 * t + 1)
                if prev is not None:
                    (pqc, pkb, pt, psp0, psp1) = prev
                    emit_expmask_pv_pair(pqc, pkb, 2 * pt, psp0)
                    emit_expmask_pv_pair(pqc, pkb, 2 * pt + 1, psp1)
                    if pkb == NB - 1 and pt == 1:
                        # normalize NOW (frees cps for the new qc) ...
                        emit_normalize_qc(pqc)
                        pending_outproj += [pqc * QC + i for i in range(QC)]
                    elif (pending_outproj and pt == 1
                          and pkb in (2, 5, 8, 11)):
                        # ... out-projection spread one qb at a time so each
                        # PE injection is small enough for the e-pool to
                        # absorb without stalling ACT
                        emit_outproj_qb(pending_outproj.pop(0))
                prev = (qc, kb, t, sp0, sp1)
    (pqc, pkb, pt, psp0, psp1) = prev
    emit_expmask_pv_pair(pqc, pkb, 2 * pt, psp0)
    emit_expmask_pv_pair(pqc, pkb, 2 * pt + 1, psp1)
    emit_normalize_qc(pqc)
    pending_outproj += [pqc * QC + i for i in range(QC)]
    for qb in pending_outproj:
        emit_outproj_qb(qb)


_CACHED = None


def _get_program():
    global _CACHED
    if _CACHED is None:
        _CACHED = build_program()
    return _CACHED


def _bf16(a):
    return np.asarray(a, dtype=ml_dtypes.bfloat16)


def _build_wo4(Wo):
    """Permute Wo rows into 4 chunks matching the fused ctx layout.

    Pair tile t = 2g+pi: row r=1..32 -> Wo row for head 4g+pi dim r-1;
    row r=65..96 -> head 4g+pi+2 dim r-65; other rows zero.
    """
    wo4 = np.zeros((4 * 128, U), dtype=np.float32)
    Wo = np.asarray(Wo, np.float32)
    for g in range(2):
        for pi in range(2):
            t = 2 * g + pi
            for jj in range(2):
                h = 4 * g + pi + 2 * jj
                wo4[t * 128 + 64 * jj + 1:t * 128 + 64 * jj + 33, :] = \
                    Wo[h * D:(h + 1) * D, :]
    return _bf16(wo4)


def kernel(node_features, adjacency_matrix, Wq, Wk, Wv, Wo, bo, **run_kwargs):
    nc = _get_program()
    xT = _bf16(np.transpose(np.asarray(node_features, np.float32), (0, 2, 1)))
    adjT = _bf16(np.transpose(np.asarray(adjacency_matrix), (0, 2, 1)))
    wo4 = _build_wo4(Wo)
    wq, wk, wv = _bf16(Wq), _bf16(Wk), _bf16(Wv)
    bo32 = np.asarray(bo, np.float32)
    in_maps = []
    for b in range(B):
        in_maps.append({
            "xT": np.ascontiguousarray(xT[b]),
            "adjT": np.ascontiguousarray(adjT[b]),
            "Wq": wq, "Wk": wk, "Wv": wv, "wo4": wo4,
            "bo": bo32,
        })
    res = run_bass_kernel_spmd(nc, in_maps, core_ids=list(range(B)), **run_kwargs)
    out = np.stack([res.results[b]["out"] for b in range(B)], axis=0)
    kernel.last_results = res
    return out
